# revision 2
# baseline (speedup 1.0000x reference)
"""GCN layer on 8 Trainium2 NeuronCores.

Computes out = A @ (x @ W.T) where A is the sparse COO adjacency
(A[r, c] = sum of edge_vals over edges (r, c)), N=100000 nodes,
E=3200000 edges, D=256.

Strategy (1D destination partition, matmul-associativity reorder):
  out = (A @ x) @ W.T
- Destination nodes are sharded across the 8 cores (12500 rows each);
  x is replicated in every core's DRAM.
- Per core, edges are grouped on the host by dest-block (128 rows) and
  laid out in 128-edge tiles.  For each tile the kernel gathers the 128
  source rows x[col[e]] with one indirect DMA (one offset per
  partition), builds a scaled one-hot selection matrix
  S[e, d] = val[e] * (rowrel[e] == d) with a single DVE tensor_scalar,
  and accumulates S.T @ xg into a PSUM tile — a segmented sum over the
  128-row dest block.  G = (A @ x)-block is then transformed by W.T on
  the tensor engine (two transposes + two accumulating matmuls) and
  written out.
- SPMD: all cores run the identical program; per-dest-block tile counts
  are padded to the max across cores (offset 0 / val 0 filler).
"""

import numpy as np

P = 128
N = 100000
E = 3200000
D = 256
NCORES = 8
SH = N // NCORES          # 12500 dest rows per core
NDB = (SH + P - 1) // P   # 98 dest blocks (last one has 84 rows)


def _prep(edge_row, edge_col, edge_vals):
    """Group edges by (core, dest-block); build per-core DMA-ready offset /
    value / dest-row tables padded uniformly across cores."""
    core = edge_row // SH
    lrow = edge_row - core * SH
    db = lrow // P
    rowrel_all = (lrow % P).astype(np.float32)
    gkey = core.astype(np.int64) * NDB + db
    order = np.argsort(gkey, kind="stable")
    col_s = edge_col[order].astype(np.int32)
    val_s = edge_vals[order]
    row_s = rowrel_all[order]

    counts = np.bincount(gkey, minlength=NCORES * NDB).reshape(NCORES, NDB)
    starts = np.zeros(NCORES * NDB + 1, np.int64)
    np.cumsum(counts.ravel(), out=starts[1:])
    max_cnt = np.maximum(counts.max(axis=0), 1)          # [NDB]
    pad_to = ((max_cnt + P - 1) // P) * P                # [NDB]
    ttot = int(pad_to.sum()) // P                        # total edge tiles

    off_hosts, val_hosts, row_hosts = [], [], []
    for m in range(NCORES):
        off_h = np.zeros((P, ttot), np.int32)
        val_h = np.zeros((P, ttot), np.float32)
        row_h = np.zeros((P, ttot), np.float32)
        toff = 0
        for dbi in range(NDB):
            p = int(pad_to[dbi])
            t = p // P
            s0 = starts[m * NDB + dbi]
            cnt = int(counts[m, dbi])
            bo = np.zeros(p, np.int32)
            bo[:cnt] = col_s[s0 : s0 + cnt]
            bv = np.zeros(p, np.float32)
            bv[:cnt] = val_s[s0 : s0 + cnt]
            br = np.zeros(p, np.float32)
            br[:cnt] = row_s[s0 : s0 + cnt]
            off_h[:, toff : toff + t] = bo.reshape(t, P).T
            val_h[:, toff : toff + t] = bv.reshape(t, P).T
            row_h[:, toff : toff + t] = br.reshape(t, P).T
            toff += t
        off_hosts.append(off_h)
        val_hosts.append(val_h)
        row_hosts.append(row_h)

    return pad_to, ttot, off_hosts, val_hosts, row_hosts


def _build(pad_to, ttot, reps=1):
    """Build the SPMD bass program (identical on all cores).  reps>1 repeats
    the whole kernel body for in-NEFF benchmarking."""
    import concourse.bacc as bacc
    import concourse.bass as bass
    import concourse.mybir as mybir
    import concourse.tile as tile

    f32 = mybir.dt.float32
    i32 = mybir.dt.int32

    nc = bacc.Bacc("TRN2")
    x_d = nc.dram_tensor("x", [N, D], f32, kind="ExternalInput")
    wt_d = nc.dram_tensor("wt", [D, D], f32, kind="ExternalInput")
    iota_d = nc.dram_tensor("iota", [P, P], f32, kind="ExternalInput")
    ident_d = nc.dram_tensor("ident", [P, P], f32, kind="ExternalInput")
    off_d = nc.dram_tensor("off", [P, ttot], i32, kind="ExternalInput")
    val_d = nc.dram_tensor("val", [P, ttot], f32, kind="ExternalInput")
    row_d = nc.dram_tensor("row", [P, ttot], f32, kind="ExternalInput")
    out_d = nc.dram_tensor("out", [SH, D], f32, kind="ExternalOutput")

    with tile.TileContext(nc) as tc:
        with (
            tc.tile_pool(name="const", bufs=1) as constp,
            tc.tile_pool(name="meta", bufs=4) as metap,
            tc.tile_pool(name="gather", bufs=24) as gatherp,
            tc.tile_pool(name="s", bufs=8) as sp,
            tc.tile_pool(name="gsb", bufs=3) as gsbp,
            tc.tile_pool(name="osb", bufs=3) as osbp,
            tc.tile_pool(name="psg", bufs=2, space="PSUM") as psg,
            tc.tile_pool(name="pst", bufs=2, space="PSUM") as pst,
            tc.tile_pool(name="pso", bufs=2, space="PSUM") as pso,
        ):
            iota_t = constp.tile([P, P], f32)
            nc.sync.dma_start(out=iota_t[:], in_=iota_d[:])
            ident_t = constp.tile([P, P], f32)
            nc.sync.dma_start(out=ident_t[:], in_=ident_d[:])
            wt_t = []
            for k in range(2):
                w = constp.tile([P, D], f32, tag=f"wt{k}")
                nc.sync.dma_start(out=w[:], in_=wt_d[k * P : (k + 1) * P, :])
                wt_t.append(w)

            for _ in range(reps):
                toff = 0
                for dbi in range(NDB):
                    t_db = int(pad_to[dbi]) // P

                    off_t = metap.tile([P, t_db], i32, tag="off")
                    nc.sync.dma_start(
                        out=off_t[:], in_=off_d[:, toff : toff + t_db]
                    )
                    val_t = metap.tile([P, t_db], f32, tag="val")
                    nc.sync.dma_start(
                        out=val_t[:], in_=val_d[:, toff : toff + t_db]
                    )
                    row_t = metap.tile([P, t_db], f32, tag="row")
                    nc.sync.dma_start(
                        out=row_t[:], in_=row_d[:, toff : toff + t_db]
                    )

                    g_ps = psg.tile([P, D], f32)
                    for t in range(t_db):
                        xg = gatherp.tile([P, D], f32)
                        nc.gpsimd.indirect_dma_start(
                            out=xg[:],
                            out_offset=None,
                            in_=x_d[:],
                            in_offset=bass.IndirectOffsetOnAxis(
                                ap=off_t[:, t : t + 1], axis=0
                            ),
                        )
                        s_t = sp.tile([P, P], f32)
                        nc.vector.tensor_scalar(
                            out=s_t[:],
                            in0=iota_t[:],
                            scalar1=row_t[:, t : t + 1],
                            scalar2=val_t[:, t : t + 1],
                            op0=mybir.AluOpType.is_equal,
                            op1=mybir.AluOpType.mult,
                        )
                        nc.tensor.matmul(
                            g_ps[:],
                            lhsT=s_t[:],
                            rhs=xg[:],
                            start=(t == 0),
                            stop=(t == t_db - 1),
                        )

                    g_sb = gsbp.tile([P, D], f32)
                    nc.vector.tensor_copy(out=g_sb[:], in_=g_ps[:])
                    o_ps = pso.tile([P, D], f32)
                    for k in range(2):
                        t_ps = pst.tile([P, P], f32)
                        nc.tensor.transpose(
                            t_ps[:], g_sb[:, k * P : (k + 1) * P], ident_t[:]
                        )
                        gt_sb = gsbp.tile([P, P], f32, tag="gt")
                        nc.vector.tensor_copy(out=gt_sb[:], in_=t_ps[:])
                        nc.tensor.matmul(
                            o_ps[:],
                            lhsT=gt_sb[:],
                            rhs=wt_t[k][:],
                            start=(k == 0),
                            stop=(k == 1),
                        )
                    o_sb = osbp.tile([P, D], f32)
                    nc.vector.tensor_copy(out=o_sb[:], in_=o_ps[:])
                    rows = min(P, SH - dbi * P)
                    nc.sync.dma_start(
                        out=out_d[dbi * P : dbi * P + rows, :],
                        in_=o_sb[:rows, :],
                    )
                    toff += t_db

    nc.compile()
    return nc


def _make_in_maps(x, W, off_hosts, val_hosts, row_hosts):
    wt = np.ascontiguousarray(W.T)
    iota = np.tile(np.arange(P, dtype=np.float32), (P, 1))
    ident = np.eye(P, dtype=np.float32)
    return [
        {
            "x": x,
            "wt": wt,
            "iota": iota,
            "ident": ident,
            "off": off_hosts[m],
            "val": val_hosts[m],
            "row": row_hosts[m],
        }
        for m in range(NCORES)
    ]


def _run(nc, in_maps):
    from concourse.bass_utils import run_bass_kernel_spmd

    res = run_bass_kernel_spmd(nc, in_maps, list(range(NCORES)))
    return np.concatenate([res.results[m]["out"] for m in range(NCORES)], axis=0)


def kernel(x, W, edge_vals, edge_row, edge_col):
    x = np.asarray(x, np.float32)
    W = np.asarray(W, np.float32)
    edge_vals = np.asarray(edge_vals, np.float32)
    edge_row = np.asarray(edge_row, np.int32)
    edge_col = np.asarray(edge_col, np.int32)

    pad_to, ttot, off_hosts, val_hosts, row_hosts = _prep(
        edge_row, edge_col, edge_vals
    )
    nc = _build(pad_to, ttot, reps=1)
    in_maps = _make_in_maps(x, W, off_hosts, val_hosts, row_hosts)
    return _run(nc, in_maps)
